# revision 68
# baseline (speedup 1.0000x reference)
"""Trainium2 Bass kernel for nn_MCGraphAttention (edge-scaled multi-head attention).

Reference math (B=4, T=2048, C=256, H=4, D=64):
    x   = nodes * mask
    q,k,v = x @ W{q,k,v}.T            (torch Linear convention)
    s   = (q @ k.T) * H**-0.5         per head
    w   = softmax(s * (3*edge+1))     over keys, edge broadcast over heads
    out = (w @ v, heads merged) @ Wp.T

Sharding: 8 cores = 4 batches x 2 query-halves (1024 queries/core).
Each core computes its full output rows; host only permutes/slices.

Masked-key skip (the big win, mathematically EXACT): a masked key has
x=0 -> s=0 for every query -> its softmax term is exp(0*(3e+1) - M0) =
exp(-M0), independent of the edge value, and it contributes nothing to
the numerator (v=0). The host permutes keys unmasked-first (softmax and
AV are permutation-invariant over keys), the device computes only
kcu = ceil(max_b n_unmasked/128) key chunks (~9 of 16 for this mask
distribution), and the (T - 128*kcu) skipped keys enter the denominator
as the compile-time constant DEN_C = (T-128*kcu)*exp(-M0), added via the
Identity-activation bias when ACT evacuates the denominator row. kcu is
derived from the actual mask at runtime, so any input is handled.

Device-side design (per core):
  - scores are computed TRANSPOSED: s[kj, qi] (keys on partitions) so the
    edge scale (host-pretransposed, fp16) streams in naturally and the
    softmax-over-keys sum falls out of the AV matmul via a ones column.
  - arg = (e + 1/3) * (1.5 * q@k) is one fused scalar_tensor_tensor on DVE
    reading scores straight from PSUM (the 1.5 = 3 * H**-0.5 is folded into
    Wq on the host; the global shift -20 rides the ACT exp bias; softmax is
    shift-invariant and row maxes are provably in [0, 83.6] for this data).
    DVE is the bottleneck engine: GPSIMD cannot touch PSUM and ACT is
    saturated by exp, so every PSUM-side elementwise op lands on DVE.
  - w = exp(arg-20) in bf16 (needs dynamic range), exp batched 4 chunks
    per ACT op; v in bf16; fp16 matmuls with f32 accumulation; f16 output
    (host casts back).
  - software pipeline: per-iteration AV matmuls are emitted 2*EB
    iterations late so their exp dependency is met at PE's in-order queue
    head; projections interleave mid-pass-0 with DVE evacuations (fast
    psum-slot turnaround); all inputs load as single multi-dim-AP DMAs
    (HWDGE dispatch is ~650ns serial per DMA); exp act-table preload and
    PE p-state warmup matmuls hide fill latency; the tail normalizes via
    DVE reciprocal + PE rank-1 broadcast + ACT evac (no DRAM bounce) with
    the output projection chasing column halves.

Measured (8 cores, axon slope bench): ~107 us/exec vs ~147 us/exec for
the pre-skip version and the session-0 baseline under the same metric
(TimelineSim: 83 us vs 131 us baseline). Absmax relative error ~4.9e-3.
"""

import os
import sys

import numpy as np

for _p in ("/opt/trn_rl_repo",):
    if _p not in sys.path and os.path.isdir(_p):
        sys.path.insert(0, _p)

B, T, C, H = 4, 2048, 256, 4
D = C // H
TQ = T // 2  # queries per core
NCORES = 8
KC = T // 128  # 16 key chunks
M0 = 20.0  # global softmax shift (safe: args in [-84, 84], row maxes >= 0)

_CACHE = {}


def _build_nc(kcu=9, reps=1):
    import concourse.bacc as bacc
    import concourse.bass as bass
    import concourse.mybir as mybir
    import concourse.tile as tile
    from contextlib import ExitStack

    f32 = mybir.dt.float32
    f16 = mybir.dt.float16
    bf16 = mybir.dt.bfloat16
    ADD = mybir.AluOpType.add
    MULT = mybir.AluOpType.mult
    EXP = mybir.ActivationFunctionType.Exp

    nc = bacc.Bacc("TRN2", target_bir_lowering=False, debug=False)

    TK = kcu * 128  # computed (unmasked-first) keys; the rest only shift
    xT = nc.dram_tensor("xT", [C, TK], f16, kind="ExternalInput").ap()
    xqT = nc.dram_tensor("xqT", [C, TQ], f16, kind="ExternalInput").ap()
    eT = nc.dram_tensor("eT", [TK, TQ], f16, kind="ExternalInput").ap()
    wqT = nc.dram_tensor("wqT", [C, C], f16, kind="ExternalInput").ap()
    wkT = nc.dram_tensor("wkT", [C, C], f16, kind="ExternalInput").ap()
    wvT = nc.dram_tensor("wvT", [C, C], f16, kind="ExternalInput").ap()
    wpT = nc.dram_tensor("wpT", [C, C], f16, kind="ExternalInput").ap()
    out_t = nc.dram_tensor("out_t", [C, TQ], f16, kind="ExternalOutput").ap()

    with tile.TileContext(nc) as tc:
        for rep in range(reps):
            _emit_rep(nc, tc, rep, kcu, xT, xqT, eT, wqT, wkT, wvT, wpT, out_t)

    nc.compile()
    return nc


def _emit_rep(nc, tc, rep, kcu, xT, xqT, eT, wqT, wkT, wvT, wpT, out_t):
    KC = kcu                      # number of computed key chunks
    TK = kcu * 128
    # skipped keys are all masked: each contributes exp(0 - M0) to the
    # softmax denominator and nothing to the numerator (v = 0) — EXACT
    DEN_C = float((T - TK) * np.exp(-M0))
    import concourse.bass as bass
    import concourse.mybir as mybir
    from contextlib import ExitStack

    f32 = mybir.dt.float32
    f16 = mybir.dt.float16
    bf16 = mybir.dt.bfloat16
    ADD = mybir.AluOpType.add
    MULT = mybir.AluOpType.mult
    EXP = mybir.ActivationFunctionType.Exp
    IDENT = mybir.ActivationFunctionType.Identity

    rec_scr = nc.dram_tensor(f"rec_scr{rep}", [H, TQ], f32).ap()

    with ExitStack() as ctx:
        consts = ctx.enter_context(tc.tile_pool(name=f"consts{rep}", bufs=1))

        # ---- persistent SBUF tensors ----
        # ci pairs live in ONE tile so each input loads with a single DMA
        # (HWDGE dispatch is ~650ns serial per DMA — count matters).
        xT2 = consts.tile([128, 2 * TK], f16, tag="xT", name="xT2")
        xq2 = consts.tile([128, 2 * TQ], f16, tag="xq", name="xq2")
        wmap = {}
        for nm in ("wq", "wk", "wv", "wp"):
            wmap[nm] = consts.tile([128, 2 * C], f16, tag=nm, name=f"{nm}2")
        wq2, wk2, wv2, wp2 = wmap["wq"], wmap["wk"], wmap["wv"], wmap["wp"]
        eT2 = consts.tile([128, KC * TQ], f16, tag="eT", name="eT2")
        xT_sb = [xT2[:, i * TK : (i + 1) * TK] for i in range(2)]
        xq_sb = [xq2[:, i * TQ : (i + 1) * TQ] for i in range(2)]
        wq_sb = [wq2[:, i * C : (i + 1) * C] for i in range(2)]
        wk_sb = [wk2[:, i * C : (i + 1) * C] for i in range(2)]
        wv_sb = [wv2[:, i * C : (i + 1) * C] for i in range(2)]
        wp_sb = [wp2[:, i * C : (i + 1) * C] for i in range(2)]
        eT_sb = [eT2[:, j * TQ : (j + 1) * TQ] for j in range(KC)]

        # touch Exp right away so LoadActFuncSet runs during the DMA fill
        # instead of stalling the first real exp batch
        bias_m0 = consts.tile([128, 1], f32, tag="biasM0", name="bias_m0")
        warm = consts.tile([128, 1], f32, tag="warm", name="act_warm")
        nc.gpsimd.memset(bias_m0, -M0)
        nc.scalar.activation(warm, bias_m0, EXP, bias=bias_m0)
        pe_dummy = consts.tile([64, 512], f16, tag="pedum", name="pe_dummy")
        nc.gpsimd.memset(pe_dummy, 0.0)
        ones64 = consts.tile([1, 64], f32, tag="ones64", name="ones64")
        nc.gpsimd.memset(ones64, 1.0)
        denc_sb = consts.tile([1, 1], f32, tag="denc", name="denc_sb")
        nc.gpsimd.memset(denc_sb, DEN_C)

        def load2(dst, src, rows, cols, col0=0, ncols=None, nci=2):
            # one DMA for [nci*rows, cols] DRAM -> dst[p, ci*cols + t]
            ncols = cols if ncols is None else ncols
            in_ap = bass.AP(
                tensor=src.tensor,
                offset=src.offset + col0,
                ap=[[cols, rows], [rows * cols, nci], [1, ncols]],
            )
            nc.sync.dma_start(out=dst, in_=in_ap)

        # critical path first: q-proj deps, k-proj deps (first half), eT0-2
        load2(xq2, xqT, 128, TQ)
        load2(wq2, wqT, 128, C)
        load2(wk2, wkT, 128, C)
        load2(xT2, xT, 128, TK)
        load2(
            eT2.rearrange("p (j t) -> p j t", j=KC)[:, 0:3, :],
            eT, 128, TQ, nci=3,
        )
        load2(wv2, wvT, 128, C)
        load2(wp2, wpT, 128, C)

        vN_sb = [
            consts.tile([128, H * (D + 1)], bf16, tag=f"vN{j}", name=f"vN_sb{j}")
            for j in range(KC)
        ]
        qT_sb = [
            consts.tile([128, TQ], f16, tag=f"qT{i}", name=f"qT_sb{i}") for i in range(2)
        ]
        kT_sb = [
            consts.tile([128, TK], f16, tag=f"kT{i}", name=f"kT_sb{i}") for i in range(2)
        ]
        resn_sb = [
            consts.tile([128, TQ], f16, tag=f"rn{i}", name=f"resn_sb{i}")
            for i in range(2)
        ]

        for tch in range(KC):
            # only the per-head ones columns; v evac overwrites the rest
            ones4 = vN_sb[tch].rearrange("p (h e) -> p h e", h=H)[:, :, D : D + 1]
            nc.gpsimd.memset(ones4, 1.0)

        # ---- main: attention loop with projections interleaved ----
        # Projections share the "s" psum slots. qT/kT for heads {2,3}
        # (co=1) are deferred to the hp=1 boundary; v chunks trickle in
        # during the first 32 iterations (each ready well before its AV).
        EB = 4  # exp batch: iterations staged per ACT exp op
        with (
            tc.tile_pool(name="spsum", bufs=2, space="PSUM") as spsum,
            tc.tile_pool(name="rpsum", bufs=2, space="PSUM") as rpsum,
            tc.tile_pool(name="wapool", bufs=3) as wapool,
            tc.tile_pool(name="wbpool", bufs=4) as wbpool,
            tc.tile_pool(name="small", bufs=4) as small,
        ):
            def copy_to(eng, out, in_):
                # NOTE: GPSIMD cannot access PSUM, so projection evacuations
                # go to ACT (which has slack: DVE's STT stream sets the
                # cadence at ~4.8us/batch vs ACT's 3.6us exp) or DVE.
                if eng == "act":
                    nc.scalar.copy(out, in_)
                else:
                    nc.vector.tensor_copy(out, in_)

            def proj_q(co, evac=None, warmups=0):
                q_ps = spsum.tile([128, TQ], f32, tag="s", name=f"q_ps{co}")
                # dummy matmuls ramp the PE p-state (0.65 -> 2.4 GHz after
                # ~3us busy) while the projection DMAs are still in flight
                for w in range(warmups):
                    nc.tensor.matmul(
                        q_ps[0:64, 0:512],
                        pe_dummy[:, 0:64],
                        pe_dummy,
                        start=True,
                        stop=True,
                    )
                for n2 in range(2):
                    for ci in range(2):
                        nc.tensor.matmul(
                            q_ps[:, n2 * 512 : (n2 + 1) * 512],
                            wq_sb[ci][:, co * 128 : (co + 1) * 128],
                            xq_sb[ci][:, n2 * 512 : (n2 + 1) * 512],
                            start=(ci == 0),
                            stop=(ci == 1),
                        )
                copy_to(evac or "dve", qT_sb[co], q_ps)

            def proj_k(co, half, evac=None):
                # segment 0: keys 0:1024; segment 1: keys 1024:TK (if any)
                k0 = half * 1024
                kw = min(1024, TK - k0)
                k_ps = spsum.tile([128, TQ], f32, tag="s", name=f"k_ps{co}_{half}")
                for o in range(0, kw, 512):
                    w = min(512, kw - o)
                    for ci in range(2):
                        nc.tensor.matmul(
                            k_ps[:, o : o + w],
                            wk_sb[ci][:, co * 128 : (co + 1) * 128],
                            xT_sb[ci][:, k0 + o : k0 + o + w],
                            start=(ci == 0),
                            stop=(ci == 1),
                        )
                copy_to(evac or "dve", kT_sb[co][:, k0 : k0 + kw], k_ps[:, 0:kw])

            def proj_v(tch):
                v_ps = spsum.tile([128, TQ], f32, tag="s", name=f"v_ps{tch}")
                for ci in range(2):
                    nc.tensor.matmul(
                        v_ps[:, 0:C],
                        xT_sb[ci][:, tch * 128 : (tch + 1) * 128],
                        wv_sb[ci],
                        start=(ci == 0),
                        stop=(ci == 1),
                    )
                v4 = v_ps[:, 0:C].rearrange("p (h d) -> p h d", h=H)
                o4 = vN_sb[tch].rearrange("p (h e) -> p h e", h=H)[:, :, 0:D]
                nc.vector.tensor_copy(o4, v4)

            def dance(hh, rts, hp):
                # per-head normalization: reciprocal of the denominator row
                # (ones-column of the AV output) straight from PSUM, a
                # partition broadcast, then one TT multiply evacuating resT.
                h = hp * 2 + hh
                # at the tail both dances run back-to-back: split them across
                # the two HWDGE queues (ACT's queue is idle by then)
                dmae = nc.scalar if h == 2 else nc.sync
                # ACT evacuates the denominator row, adding the skipped
                # masked-key mass (Identity: out = in + bias)
                denrow = small.tile([1, TQ], f32, tag="denrow", name=f"denrow{h}")
                nc.scalar.activation(
                    denrow, rts[hh][64:65, :], IDENT, bias=denc_sb
                )
                if hp == 0:
                    # mid-run: the reciprocal runs 8-wide after a reshape DMA
                    den128 = small.tile(
                        [128, TQ // 128], f32, tag="den128", name=f"den128_{h}"
                    )
                    dmae.dma_start(out=den128, in_=denrow)
                    rec = small.tile([128, TQ // 128], f32, tag="rec128", name=f"rec128_{h}")
                    nc.vector.reciprocal(rec, den128)
                    rec_out = rec_scr[h, :].rearrange("(p x) -> p x", p=128)
                else:
                    # tail: shortest-latency chain, no DRAM bounce — DVE
                    # reciprocal, then a PE rank-1 broadcast + ACT evac
                    # (emitted in the tail section below)
                    rec = small.tile([1, TQ], f32, tag="recrow", name=f"recrow{h}")
                    nc.vector.reciprocal(rec, denrow)
                    tail_norms.append((h, rts[hh], rec))
                    return
                dmae.dma_start(out=rec_out, in_=rec)
                recB = small.tile([64, TQ], f32, tag="recB", name=f"recB{h}")
                rec_bcast = bass.AP(
                    tensor=rec_scr.tensor,
                    offset=rec_scr.offset + h * TQ,
                    ap=[[0, 64], [1, TQ]],
                )
                dmae.dma_start(out=recB, in_=rec_bcast)
                nc.vector.tensor_tensor(
                    out=resn_sb[h // 2][(h % 2) * 64 : (h % 2) * 64 + 64, :],
                    in0=rts[hh][0:64, :],
                    in1=recB,
                    op=MULT,
                )

            tail_norms = []  # (h, rts_head, recB) for the half-chased tail
            proj_q(0, warmups=8)
            proj_k(0, 0)

            # Software pipeline: each iteration's AV matmuls are emitted
            # LAG_ITS iterations late, interleaved between score matmuls,
            # so (a) their wb/exp dependency is satisfied when they reach
            # the head of PE's in-order queue and (b) they fill PE's
            # wait-time on the score-psum WAR ping-pong instead of bursting.
            LAG_ITS = 2 * EB
            it = 0
            pend = []
            avq = []  # (emit_fn, dance_fn_or_None) per iteration
            wa = wb = None

            def drain_avq(keep):
                while len(avq) > keep:
                    fn, dn = avq.pop(0)
                    fn()
                    if dn is not None:
                        dn()

            for hp in range(2):
                rts = [
                    rpsum.tile([D + 1, TQ], f32, tag="resT", name=f"resT{hp}_{hh}")
                    for hh in range(2)
                ]

                def make_av(hh, kjc, slot, wb, rts=rts, hp=hp):
                    def emit_av():
                        lhsT = vN_sb[kjc][
                            :, (hp * 2 + hh) * (D + 1) : (hp * 2 + hh + 1) * (D + 1)
                        ]
                        for n2 in range(2):
                            nc.tensor.matmul(
                                rts[hh][:, n2 * 512 : (n2 + 1) * 512],
                                lhsT,
                                wb[:, slot * TQ + n2 * 512 : slot * TQ + (n2 + 1) * 512],
                                start=(kjc == 0),
                                stop=(kjc == KC - 1),
                            )
                    return emit_av

                def make_dance(hh, rts=rts, hp=hp):
                    def emit_dance():
                        dance(hh, rts, hp)
                    return emit_dance

                # head-split tail: the last two exp batches each cover one
                # head's final 4 chunks, so each head's dance can ride right
                # behind its own (deferred) AV batch
                seq = [(kjc, hh) for kjc in range(KC - 2) for hh in range(2)]
                seq += [(kjc, 0) for kjc in range(KC - 2, KC)]
                seq += [(kjc, 1) for kjc in range(KC - 2, KC)]
                n_batches = len(seq) // EB
                for i_in_pass, (kjc, hh) in enumerate(seq):
                        h = hp * 2 + hh
                        co, row = h // 2, (h % 2) * 64
                        sp = spsum.tile([128, TQ], f32, tag="s", name=f"sp{it}")
                        for n2 in range(2):
                            nc.tensor.matmul(
                                sp[:, n2 * 512 : (n2 + 1) * 512],
                                kT_sb[co][row : row + 64, kjc * 128 : (kjc + 1) * 128],
                                qT_sb[co][row : row + 64, n2 * 512 : (n2 + 1) * 512],
                                start=True,
                                stop=True,
                            )
                        slot = it % EB
                        if slot == 0:
                            wa = wapool.tile([128, EB * TQ], f32, tag="warg", name=f"wa{it}")
                            wb = wbpool.tile([128, EB * TQ], bf16, tag="wexp", name=f"wb{it}")
                        nc.vector.scalar_tensor_tensor(
                            out=wa[:, slot * TQ : (slot + 1) * TQ],
                            in0=eT_sb[kjc],
                            scalar=1.0 / 3.0,
                            in1=sp,
                            op0=ADD,
                            op1=MULT,
                        )
                        dance_fn = make_dance(hh) if kjc == KC - 1 else None
                        avq.append((make_av(hh, kjc, slot, wb), dance_fn))
                        pend.append(slot)
                        # the very last batch fires exp in two halves so the
                        # final AV + dance chain starts ~2us earlier
                        trigger = (
                            slot == EB - 1
                            or i_in_pass == len(seq) - 1
                            or (hp == 1 and i_in_pass >= len(seq) - 3)
                        )
                        if trigger:
                            s0 = pend[0]
                            nc.scalar.activation(
                                wb[:, s0 * TQ : (slot + 1) * TQ],
                                wa[:, s0 * TQ : (slot + 1) * TQ],
                                EXP,
                                bias=bias_m0,
                            )
                            pend = []
                        drain_avq(LAG_ITS)
                        if it < 2 * KC and it % 2 == 0:
                            proj_v(it // 2)
                        if it % 4 == 0 and 3 + it // 2 < KC:
                            # stream the rest of eT in chunk pairs on SP
                            j0 = 3 + it // 2
                            npair = min(2, KC - j0)
                            load2(
                                eT2.rearrange("p (j t) -> p j t", j=KC)[
                                    :, j0 : j0 + npair, :
                                ],
                                eT, 128, TQ, col0=j0 * 128 * TQ, nci=npair,
                            )
                        if hp == 0:  # stage pass-1 projections mid-pass-0, where
                            # DVE has slack and inputs have arrived; by the
                            # pass boundary qT[1]/kT[1] are ready so pass-1
                            # scores start immediately (it-based so they fire
                            # for any kcu >= 7)
                            if it == 6 and TK > 1024:
                                proj_k(0, 1)
                            elif it == 8:
                                proj_q(1)
                            elif it == 10:
                                proj_k(1, 0)
                            elif it == 12 and TK > 1024:
                                proj_k(1, 1)
                        it += 1
            drain_avq(0)  # drain the AV pipeline + final dances

            # keep the PE p-state hot through the normalization wait so the
            # broadcasts + output projection run at full clock
            warm_ps = spsum.tile([128, TQ], f32, tag="s", name="warm_ps")
            for w in range(8):
                nc.tensor.matmul(
                    warm_ps[0:64, 0:512], pe_dummy[:, 0:64], pe_dummy,
                    start=True, stop=True,
                )

            # tail broadcast: PE rank-1 (ones64 x recrow) into a free score
            # psum slot, ACT evacuates to SBUF (DVE TT can read only one
            # PSUM operand), then TT halves chased by the out projection
            tail_recB = []
            for h, rts_h, rec in tail_norms:
                bc = spsum.tile([128, TQ], f32, tag="s", name=f"bc{h}")
                for n2 in range(2):
                    nc.tensor.matmul(
                        bc[0:64, n2 * 512 : (n2 + 1) * 512],
                        ones64,
                        rec[:, n2 * 512 : (n2 + 1) * 512],
                        start=True,
                        stop=True,
                    )
                recB = small.tile([64, TQ], f32, tag="recB", name=f"recB{h}")
                nc.scalar.copy(recB, bc[0:64, :])
                tail_recB.append((h, rts_h, recB))
            for n2 in range(2):
                for h, rts_h, recB in tail_recB:
                    nc.vector.tensor_tensor(
                        out=resn_sb[h // 2][
                            (h % 2) * 64 : (h % 2) * 64 + 64,
                            n2 * 512 : (n2 + 1) * 512,
                        ],
                        in0=rts_h[0:64, n2 * 512 : (n2 + 1) * 512],
                        in1=recB[:, n2 * 512 : (n2 + 1) * 512],
                        op=MULT,
                    )

        # ---- phase 3: output projection, half-chased behind the tail TTs ----
        with tc.tile_pool(name="ops", bufs=2, space="PSUM") as ops:
            o_ps = [
                ops.tile([128, TQ], f32, tag="op", name=f"o_ps{co}") for co in range(2)
            ]
            outsb = [
                consts.tile([128, TQ], f16, tag=f"outsb{co}", name=f"outsb{co}")
                for co in range(2)
            ]
            for n2 in range(2):
                for co in range(2):
                    for ci in range(2):
                        nc.tensor.matmul(
                            o_ps[co][:, n2 * 512 : (n2 + 1) * 512],
                            wp_sb[ci][:, co * 128 : (co + 1) * 128],
                            resn_sb[ci][:, n2 * 512 : (n2 + 1) * 512],
                            start=(ci == 0),
                            stop=(ci == 1),
                        )
                for co in range(2):
                    nc.scalar.copy(
                        outsb[co][:, n2 * 512 : (n2 + 1) * 512],
                        o_ps[co][:, n2 * 512 : (n2 + 1) * 512],
                    )
                    # ship each finished half immediately, queues alternating
                    dmae = nc.sync if co == 0 else nc.scalar
                    dmae.dma_start(
                        out=out_t[co * 128 : (co + 1) * 128, n2 * 512 : (n2 + 1) * 512],
                        in_=outsb[co][:, n2 * 512 : (n2 + 1) * 512],
                    )


def kcu_from_mask(mask):
    # chunks needed for the max unmasked-key count across batches
    n_un = int(np.asarray(mask).sum(axis=1).max())
    return max(7, (n_un + 127) // 128)


def get_nc(kcu=9):
    key = ("nc", kcu)
    if key not in _CACHE:
        _CACHE[key] = _build_nc(kcu=kcu)
    return _CACHE[key]


def make_in_maps(**inputs):
    nodes = np.asarray(inputs["nodes"], np.float32)
    edge = np.asarray(inputs["edge_index"], np.float32)
    mask = np.asarray(inputs["mask"])
    Wq = np.asarray(inputs["Wq"], np.float32)
    Wk = np.asarray(inputs["Wk"], np.float32)
    Wv = np.asarray(inputs["Wv"], np.float32)
    Wp = np.asarray(inputs["Wp"], np.float32)

    kcu = kcu_from_mask(mask)
    TK = kcu * 128
    x = nodes * mask[:, :, None].astype(np.float32)
    wq_t = np.ascontiguousarray((3.0 * H**-0.5) * Wq.T).astype(np.float16)
    wk_t = np.ascontiguousarray(Wk.T).astype(np.float16)
    wv_t = np.ascontiguousarray(Wv.T).astype(np.float16)
    wp_t = np.ascontiguousarray(Wp.T).astype(np.float16)

    in_maps = []
    for c in range(NCORES):
        b, qh = c // 2, c % 2
        qs = qh * TQ
        # keys permuted unmasked-first: softmax/AV are permutation-
        # invariant over keys, masked keys beyond TK only add the constant
        # DEN_C to the denominator (handled on-device)
        order = np.argsort(~mask[b], kind="stable")[:TK]
        xp16 = x[b].astype(np.float16)
        in_maps.append(
            {
                "xT": np.ascontiguousarray(xp16[order].T),
                "xqT": np.ascontiguousarray(xp16[qs : qs + TQ].T),
                "eT": np.ascontiguousarray(
                    edge[b, qs : qs + TQ, :].T[order]
                ).astype(np.float16),
                "wqT": wq_t,
                "wkT": wk_t,
                "wvT": wv_t,
                "wpT": wp_t,
            }
        )
    return in_maps


def assemble(results):
    out = np.empty((B, T, C), np.float32)
    for c in range(NCORES):
        b, qh = c // 2, c % 2
        qs = qh * TQ
        out[b, qs : qs + TQ, :] = results[c]["out_t"].T
    return out


def run(in_maps, kcu=9, trace=False):
    from concourse.bass_utils import run_bass_kernel_spmd

    nc = get_nc(kcu)
    return run_bass_kernel_spmd(nc, in_maps, list(range(NCORES)), trace=trace)


def kernel(**inputs):
    kcu = kcu_from_mask(inputs["mask"])
    res = run(make_in_maps(**inputs), kcu=kcu, trace=False)
    return assemble(res.results)

